# revision 9
# baseline (speedup 1.0000x reference)
"""Single-head attention (B=4, S=4096, D=A=1024, fp32 I/O) on 8 TRN2 NeuronCores.

Sharding: core c handles batch b=c//2, sequence-half h=c%2 (2048 rows).
Each core projects Q, K^T and V for its own half only; core pairs exchange
K^T/V halves with chunked AllGathers (overlapped with projection compute), so
nothing is computed twice.  Attention then runs flash-style per 512-query
block against the full gathered sequence.

Device layout is transpose-free: host passes x[b]^T slices and pre-transposed
weights; Q^T,K^T live as [A,S] (a on partitions), V as [S,A] (k on
partitions); scores are computed transposed ([k,q]); softmax normalization is
deferred to the output projection epilogue (exp without max subtraction is
safe here: scores are O(5)).  Matmul compute in bf16, accumulation fp32.
k-tiles are enumerated in gather order everywhere, which keeps scores, exp,
sums and ctx consistent without any index remapping.
"""

import numpy as np
import ml_dtypes

import concourse.bass as bass
import concourse.tile as tile
from concourse import mybir
from concourse.bass_utils import run_bass_kernel_spmd

BF = mybir.dt.bfloat16
F32 = mybir.dt.float32
AF = mybir.ActivationFunctionType

B, S, DIM, A = 4, 4096, 1024, 1024
SQ = S // 2          # rows handled per core (query rows and local K/V rows)
NC = DIM // 128      # d chunks
NA = A // 128        # a tiles
NK = S // 128        # k tiles (global)
QB = 512             # q block width
NQB = SQ // QB
SCALE = 1.0 / np.sqrt(np.float32(A))

N_CORES = 8
PAIRS = [[0, 1], [2, 3], [4, 5], [6, 7]]

LAST_RESULT = None   # BassKernelResults of the most recent run (for test.py)


def _split_multiwaits(nc):
    """This walrus build rejects instructions carrying more than one sem wait
    (and Drains carrying any); hoist extra waits into single-wait NoOps
    preceding the instruction on the same engine."""
    for f in nc.m.functions:
        for bb in f.blocks:
            new_insts = []
            for inst in bb.instructions:
                si = inst.sync_info
                if si is not None and si.on_wait:
                    keep = 0 if isinstance(inst, mybir.InstDrain) else 1
                    if len(si.on_wait) > keep:
                        waits = list(si.on_wait)
                        hoist, rest = waits[: len(waits) - keep], waits[len(waits) - keep :]
                        for w in hoist:
                            nop = mybir.InstNoOp(
                                name=nc.get_next_instruction_name(),
                                sync_info=mybir.SyncInfo(on_wait=[w], on_update=[]),
                                bass_nofuse=True,
                                engine=inst.engine,
                            )
                            nc.register_instruction(nop)
                            new_insts.append(nop)
                        si.on_wait.clear()
                        si.on_wait.extend(rest)
                new_insts.append(inst)
            bb.instructions[:] = new_insts


def _build():
    nc = bass.Bass()

    xTq = nc.declare_dram_parameter("xTq", [DIM, SQ], BF, isOutput=False)
    WqT = nc.declare_dram_parameter("WqT", [DIM, A], BF, isOutput=False)
    WkT = nc.declare_dram_parameter("WkT", [DIM, A], BF, isOutput=False)
    WvT = nc.declare_dram_parameter("WvT", [DIM, A], BF, isOutput=False)
    WoT = nc.declare_dram_parameter("WoT", [A, DIM], BF, isOutput=False)
    bqc = nc.declare_dram_parameter("bqc", [128, NA], F32, isOutput=False)
    bkc = nc.declare_dram_parameter("bkc", [128, NA], F32, isOutput=False)
    bvb = nc.declare_dram_parameter("bvb", [128, A], F32, isOutput=False)
    bob = nc.declare_dram_parameter("bob", [128, DIM], F32, isOutput=False)
    out = nc.declare_dram_parameter("out", [SQ, DIM], F32, isOutput=True)

    with tile.TileContext(nc) as tc:
        with (
            tc.tile_pool(name="dram", bufs=1, space="DRAM") as dram,
            tc.tile_pool(name="singles", bufs=1) as singles,
        ):
            QT_d = dram.tile([A, SQ], BF)
            # per-chunk collective buffers: local 1024 cols/rows -> gathered 2x
            kt_in = [
                dram.tile([A, 1024], BF, name=f"kt_in{c}", tag=f"kti{c}")
                for c in range(2)
            ]
            kt_out = [
                dram.tile([2, A, 1024], BF, name=f"kt_out{c}", tag=f"kto{c}")
                for c in range(2)
            ]
            v_in = [
                dram.tile([1024, A], BF, name=f"v_in{c}", tag=f"vi{c}")
                for c in range(2)
            ]
            v_out = [
                dram.tile([2, 1024, A], BF, name=f"v_out{c}", tag=f"vo{c}")
                for c in range(2)
            ]

            cc_warm_in = dram.tile([1, 128], BF, name="cc_warm_in")
            cc_warm_out = dram.tile([2, 1, 128], BF, name="cc_warm_out")

            v_sb = singles.tile([128, NK, A], BF)        # V resident, 8.4 MB
            wo_sb = singles.tile([128, NC, DIM], BF)     # WoT, 2.1 MB
            bqc_sb = singles.tile([128, NA], F32)
            bkc_sb = singles.tile([128, NA], F32)
            bvb_sb = singles.tile([128, A], F32)
            bob_sb = singles.tile([128, DIM], F32)
            ones_k = singles.tile([128, 1], BF)          # sums matmul lhsT
            ones_1 = singles.tile([1, 1], F32)           # row->partition matmul rhs

            # ---------------- Phase 1: projections + K/V exchange ----------
            with (
                tc.tile_pool(name="p1w", bufs=1) as p1w,
                tc.tile_pool(name="p1x", bufs=3) as p1x,
                tc.tile_pool(name="p1o", bufs=4) as p1o,
                tc.tile_pool(name="p1pk", bufs=2, space="PSUM") as p1pk,
                tc.tile_pool(name="p1pv", bufs=2, space="PSUM") as p1pv,
            ):
                wq = p1w.tile([128, NC, A], BF, tag="wq")
                wk = p1w.tile([128, NC, A], BF, tag="wk")
                wv = p1w.tile([128, NC, A], BF, tag="wv")

                def load_xs(sb, eng=None):
                    xs = p1x.tile([128, NC, 512], BF)
                    (eng or nc.sync).dma_start(
                        out=xs[:],
                        in_=xTq[:, sb * 512 : (sb + 1) * 512].rearrange(
                            "(c p) s -> p c s", p=128
                        ),
                    )
                    return xs

                # minimal DMA before the first matmul: wk + first x block,
                # halves spread across queues so dc=0 matmuls start early
                nc.sync.dma_start(
                    out=wk[:, 0:4, :],
                    in_=WkT[:, :].rearrange("(c p) a -> p c a", p=128)[:, 0:4, :],
                )
                nc.scalar.dma_start(
                    out=wk[:, 4:8, :],
                    in_=WkT[:, :].rearrange("(c p) a -> p c a", p=128)[:, 4:8, :],
                )
                nc.scalar.dma_start(out=bkc_sb[:], in_=bkc[:])
                xs_next = load_xs(0, eng=nc.gpsimd)

                def kt_chunk(c):
                    nonlocal xs_next
                    for sbl in range(2):
                        sb = c * 2 + sbl
                        xs = xs_next
                        xs_next = load_xs((sb + 1) % 4)
                        for am in range(NA):
                            pk = p1pk.tile([128, 512], F32)
                            for dc in range(NC):
                                nc.tensor.matmul(
                                    pk[:],
                                    lhsT=wk[:, dc, am * 128 : (am + 1) * 128],
                                    rhs=xs[:, dc, :],
                                    start=(dc == 0),
                                    stop=(dc == NC - 1),
                                )
                            ko = p1o.tile([128, 512], BF)
                            nc.scalar.activation(
                                ko[:], pk[:], AF.Identity, bias=bkc_sb[:, am : am + 1]
                            )
                            nc.sync.dma_start(
                                out=kt_in[c][
                                    am * 128 : (am + 1) * 128,
                                    sbl * 512 : (sbl + 1) * 512,
                                ],
                                in_=ko[:],
                            )
                    nc.gpsimd.collective_compute(
                        "AllGather",
                        mybir.AluOpType.bypass,
                        replica_groups=PAIRS,
                        ins=[kt_in[c][:].opt()],
                        outs=[kt_out[c][:].opt()],
                    )

                def v_chunk(c):
                    nonlocal xs_next
                    for sbl in range(2):
                        sb = c * 2 + sbl
                        xs = xs_next
                        xs_next = load_xs((sb + 1) % 4)
                        for st in range(4):
                            pv = p1pv.tile([128, 1024], F32)
                            for half in range(2):
                                for dc in range(NC):
                                    nc.tensor.matmul(
                                        pv[:, half * 512 : (half + 1) * 512],
                                        lhsT=xs[:, dc, st * 128 : (st + 1) * 128],
                                        rhs=wv[:, dc, half * 512 : (half + 1) * 512],
                                        start=(dc == 0),
                                        stop=(dc == NC - 1),
                                    )
                            vo = p1o.tile([128, 1024], BF, tag="vo")
                            nc.vector.tensor_add(vo[:], pv[:], bvb_sb[:])
                            nc.scalar.dma_start(
                                out=v_in[c][
                                    (sbl * 4 + st) * 128 : (sbl * 4 + st + 1) * 128, :
                                ],
                                in_=vo[:],
                            )
                    nc.gpsimd.collective_compute(
                        "AllGather",
                        mybir.AluOpType.bypass,
                        replica_groups=PAIRS,
                        ins=[v_in[c][:].opt()],
                        outs=[v_out[c][:].opt()],
                    )

                # K^T chunks first so the exchanges start as early as possible
                kt_chunk(0)
                nc.sync.dma_start(out=wv[:], in_=WvT.rearrange("(c p) a -> p c a", p=128))
                nc.scalar.dma_start(out=bvb_sb[:], in_=bvb[:])
                kt_chunk(1)
                v_chunk(0)
                nc.sync.dma_start(out=wq[:], in_=WqT.rearrange("(c p) a -> p c a", p=128))
                nc.scalar.dma_start(out=bqc_sb[:], in_=bqc[:])
                v_chunk(1)

                # gathered V -> resident SBUF, k enumerated in gather order
                for c in range(2):
                    for hh in range(2):
                        nc.gpsimd.dma_start(
                            out=v_sb[:, c * 16 + hh * 8 : c * 16 + hh * 8 + 8, :],
                            in_=v_out[c][hh].rearrange("(j p) a -> p j a", p=128),
                        )

                # --- Q projection (overlaps the V exchanges) ---
                for qb in range(NQB):
                    xs = xs_next
                    if qb < 3:
                        xs_next = load_xs(qb + 1)
                    for am in range(NA):
                        pq = p1pk.tile([128, 512], F32)
                        for dc in range(NC):
                            nc.tensor.matmul(
                                pq[:],
                                lhsT=wq[:, dc, am * 128 : (am + 1) * 128],
                                rhs=xs[:, dc, :],
                                start=(dc == 0),
                                stop=(dc == NC - 1),
                            )
                        qo = p1o.tile([128, 512], BF)
                        nc.scalar.activation(
                            qo[:], pq[:], AF.Identity, bias=bqc_sb[:, am : am + 1]
                        )
                        nc.scalar.dma_start(
                            out=QT_d[am * 128 : (am + 1) * 128, qb * 512 : (qb + 1) * 512],
                            in_=qo[:],
                        )

                # singles needed only for phase 2
                nc.sync.dma_start(
                    out=wo_sb[:], in_=WoT.rearrange("(c p) d -> p c d", p=128)
                )
                nc.sync.dma_start(out=bob_sb[:], in_=bob[:])
                nc.vector.memset(ones_k[:], 1.0)
                nc.vector.memset(ones_1[:], 1.0)

            # ---------------- Phase 2: attention ----------------
            with (
                tc.tile_pool(name="p2q", bufs=2) as p2q,
                tc.tile_pool(name="p2k", bufs=2) as p2k,
                tc.tile_pool(name="p2e", bufs=2) as p2e,
                tc.tile_pool(name="p2c", bufs=1) as p2c,
                tc.tile_pool(name="p2s", bufs=2) as p2s,
                tc.tile_pool(name="p2r", bufs=2) as p2r,
                tc.tile_pool(name="p2o", bufs=2) as p2o,
                tc.tile_pool(name="pps", bufs=2, space="PSUM") as pps,
                tc.tile_pool(name="ppsum", bufs=1, space="PSUM") as ppsum,
                tc.tile_pool(name="ppt", bufs=1, space="PSUM") as ppt,
                tc.tile_pool(name="ppc", bufs=2, space="PSUM") as ppc,
                tc.tile_pool(name="ppo", bufs=2, space="PSUM") as ppo,
            ):
                for qb in range(NQB):
                    qt = p2q.tile([128, NC, QB], BF)
                    nc.sync.dma_start(
                        out=qt[:],
                        in_=QT_d[:, qb * QB : (qb + 1) * QB].rearrange(
                            "(c p) q -> p c q", p=128
                        ),
                    )
                    et = p2e.tile([128, NK, QB], BF)
                    # scores^T + exp; k-tile groups of 4 share one KT load
                    for c in range(2):
                        for hh in range(2):
                            for half in range(2):
                                ks = p2k.tile([128, NC, 512], BF)
                                nc.sync.dma_start(
                                    out=ks[:],
                                    in_=kt_out[c][
                                        hh, :, half * 512 : (half + 1) * 512
                                    ].rearrange("(cp p) k -> p cp k", p=128),
                                )
                                ebase = c * 16 + hh * 8 + half * 4
                                for kt4 in range(4):
                                    ps = pps.tile([128, QB], F32)
                                    for ac in range(NC):
                                        nc.tensor.matmul(
                                            ps[:],
                                            lhsT=ks[:, ac, kt4 * 128 : (kt4 + 1) * 128],
                                            rhs=qt[:, ac, :],
                                            start=(ac == 0),
                                            stop=(ac == NC - 1),
                                        )
                                    nc.scalar.activation(
                                        et[:, ebase + kt4, :],
                                        ps[:],
                                        AF.Exp,
                                        scale=float(SCALE),
                                    )
                    # softmax denominators: ones-row matmul, then row->partition
                    p_row = ppsum.tile([1, QB], F32)
                    for kt in range(NK):
                        nc.tensor.matmul(
                            p_row[:],
                            lhsT=ones_k[:, 0:1],
                            rhs=et[:, kt, :],
                            start=(kt == 0),
                            stop=(kt == NK - 1),
                        )
                    srow = p2s.tile([1, QB], F32)
                    nc.scalar.copy(srow[:], p_row[:])
                    recips = p2r.tile([128, 4], F32)
                    for qi in range(4):
                        ptt = ppt.tile([128, 1], F32)
                        nc.tensor.matmul(
                            ptt[:],
                            lhsT=srow[0:1, qi * 128 : (qi + 1) * 128],
                            rhs=ones_1[0:1, 0:1],
                            start=True,
                            stop=True,
                        )
                        nc.vector.reciprocal(recips[:, qi : qi + 1], ptt[:])
                    # unnormalized ctx^T accumulated over k
                    ct = p2c.tile([128, NA, QB], BF)
                    for at in range(NA):
                        pc = ppc.tile([128, QB], F32)
                        for kt in range(NK):
                            nc.tensor.matmul(
                                pc[:],
                                lhsT=v_sb[:, kt, at * 128 : (at + 1) * 128],
                                rhs=et[:, kt, :],
                                start=(kt == 0),
                                stop=(kt == NK - 1),
                            )
                        nc.vector.tensor_copy(ct[:, at, :], pc[:])
                    # output projection + deferred softmax normalization + bias
                    for qi in range(4):
                        for half in range(2):
                            po = ppo.tile([128, 512], F32)
                            for ac in range(NC):
                                nc.tensor.matmul(
                                    po[:],
                                    lhsT=ct[:, ac, qi * 128 : (qi + 1) * 128],
                                    rhs=wo_sb[:, ac, half * 512 : (half + 1) * 512],
                                    start=(ac == 0),
                                    stop=(ac == NC - 1),
                                )
                            ob = p2o.tile([128, 512], F32)
                            nc.vector.tensor_scalar(
                                ob[:],
                                po[:],
                                recips[:, qi : qi + 1],
                                None,
                                op0=mybir.AluOpType.mult,
                            )
                            nc.vector.tensor_add(
                                ob[:], ob[:], bob_sb[:, half * 512 : (half + 1) * 512]
                            )
                            nc.sync.dma_start(
                                out=out[
                                    (qb * 4 + qi) * 128 : (qb * 4 + qi + 1) * 128,
                                    half * 512 : (half + 1) * 512,
                                ],
                                in_=ob[:],
                            )

    _split_multiwaits(nc)
    return nc


_NC_CACHE = None


def _get_nc():
    global _NC_CACHE
    if _NC_CACHE is None:
        _NC_CACHE = _build()
    return _NC_CACHE


def kernel(x, Wq, bq, Wk, bk, Wv, bv, Wo, bo):
    global LAST_RESULT
    bf16 = ml_dtypes.bfloat16
    x = np.asarray(x, np.float32)

    WqT = np.ascontiguousarray(np.asarray(Wq, np.float32).T).astype(bf16)
    WkT = np.ascontiguousarray(np.asarray(Wk, np.float32).T).astype(bf16)
    WvT = np.ascontiguousarray(np.asarray(Wv, np.float32).T).astype(bf16)
    WoT = np.ascontiguousarray(np.asarray(Wo, np.float32).T).astype(bf16)
    bqc = np.ascontiguousarray(np.asarray(bq, np.float32).reshape(NA, 128).T)
    bkc = np.ascontiguousarray(np.asarray(bk, np.float32).reshape(NA, 128).T)
    bvb = np.ascontiguousarray(np.broadcast_to(np.asarray(bv, np.float32), (128, A)))
    bob = np.ascontiguousarray(np.broadcast_to(np.asarray(bo, np.float32), (128, DIM)))

    in_maps = []
    for c in range(N_CORES):
        b, h = c // 2, c % 2
        xTq = np.ascontiguousarray(x[b, h * SQ : (h + 1) * SQ, :].T).astype(bf16)
        in_maps.append(
            {
                "xTq": xTq,
                "WqT": WqT,
                "WkT": WkT,
                "WvT": WvT,
                "WoT": WoT,
                "bqc": bqc,
                "bkc": bkc,
                "bvb": bvb,
                "bob": bob,
            }
        )

    nc = _get_nc()
    import os

    res = run_bass_kernel_spmd(
        nc,
        in_maps,
        core_ids=list(range(N_CORES)),
        trace=bool(os.environ.get("BASS_TRACE")),
    )
    LAST_RESULT = res

    out_full = np.empty((B, S, DIM), np.float32)
    for c in range(N_CORES):
        b, h = c // 2, c % 2
        out_full[b, h * SQ : (h + 1) * SQ, :] = res.results[c]["out"]
    return out_full


# revision 11
# speedup vs baseline: 1.0666x; 1.0666x over previous
"""Single-head attention (B=4, S=4096, D=A=1024, fp32 I/O) on 8 TRN2 NeuronCores.

Sharding: core c handles batch b=c//2, sequence-half h=c%2 (2048 rows).
Each core projects Q, K^T and V for its own half only; core pairs exchange
K^T/V halves with chunked AllGathers (overlapped with projection compute), so
nothing is computed twice.  Attention then runs flash-style per 512-query
block against the full gathered sequence.

Device layout is transpose-free: host passes x[b]^T slices and pre-transposed
weights; Q^T,K^T live as [A,S] (a on partitions), V as [S,A] (k on
partitions); scores are computed transposed ([k,q]); softmax normalization is
deferred to the output projection epilogue (exp without max subtraction is
safe here: scores are O(5)).  Matmul compute in bf16, accumulation fp32.
k-tiles are enumerated in gather order everywhere, which keeps scores, exp,
sums and ctx consistent without any index remapping.
"""

import numpy as np
import ml_dtypes

import concourse.bass as bass
import concourse.tile as tile
from concourse import mybir
from concourse.bass_utils import run_bass_kernel_spmd

BF = mybir.dt.bfloat16
F32 = mybir.dt.float32
AF = mybir.ActivationFunctionType

B, S, DIM, A = 4, 4096, 1024, 1024
SQ = S // 2          # rows handled per core (query rows and local K/V rows)
NC = DIM // 128      # d chunks
NA = A // 128        # a tiles
NK = S // 128        # k tiles (global)
QB = 512             # q block width
NQB = SQ // QB
SCALE = 1.0 / np.sqrt(np.float32(A))

N_CORES = 8
PAIRS = [[0, 1], [2, 3], [4, 5], [6, 7]]

LAST_RESULT = None   # BassKernelResults of the most recent run (for test.py)


def _split_multiwaits(nc):
    """This walrus build rejects instructions carrying more than one sem wait
    (and Drains carrying any); hoist extra waits into single-wait NoOps
    preceding the instruction on the same engine."""
    for f in nc.m.functions:
        for bb in f.blocks:
            new_insts = []
            for inst in bb.instructions:
                si = inst.sync_info
                if si is not None and si.on_wait:
                    keep = 0 if isinstance(inst, mybir.InstDrain) else 1
                    if len(si.on_wait) > keep:
                        waits = list(si.on_wait)
                        hoist, rest = waits[: len(waits) - keep], waits[len(waits) - keep :]
                        for w in hoist:
                            nop = mybir.InstNoOp(
                                name=nc.get_next_instruction_name(),
                                sync_info=mybir.SyncInfo(on_wait=[w], on_update=[]),
                                bass_nofuse=True,
                                engine=inst.engine,
                            )
                            nc.register_instruction(nop)
                            new_insts.append(nop)
                        si.on_wait.clear()
                        si.on_wait.extend(rest)
                new_insts.append(inst)
            bb.instructions[:] = new_insts


def _build():
    nc = bass.Bass()

    xTq = nc.declare_dram_parameter("xTq", [DIM, SQ], BF, isOutput=False)
    WqT = nc.declare_dram_parameter("WqT", [DIM, A], BF, isOutput=False)
    WkT = nc.declare_dram_parameter("WkT", [DIM, A], BF, isOutput=False)
    WvT = nc.declare_dram_parameter("WvT", [DIM, A], BF, isOutput=False)
    WoT = nc.declare_dram_parameter("WoT", [A, DIM], BF, isOutput=False)
    bqc = nc.declare_dram_parameter("bqc", [128, NA], F32, isOutput=False)
    bkc = nc.declare_dram_parameter("bkc", [128, NA], F32, isOutput=False)
    bvb = nc.declare_dram_parameter("bvb", [128, A], F32, isOutput=False)
    bob = nc.declare_dram_parameter("bob", [128, DIM], F32, isOutput=False)
    out = nc.declare_dram_parameter("out", [SQ, DIM], F32, isOutput=True)

    with tile.TileContext(nc) as tc:
        with (
            tc.tile_pool(name="dram", bufs=1, space="DRAM") as dram,
            tc.tile_pool(name="singles", bufs=1) as singles,
        ):
            # per-chunk collective buffers: local 1024 cols/rows -> gathered 2x
            kt_in = [
                dram.tile([A, 1024], BF, name=f"kt_in{c}", tag=f"kti{c}")
                for c in range(2)
            ]
            kt_out = [
                dram.tile([2, A, 1024], BF, name=f"kt_out{c}", tag=f"kto{c}")
                for c in range(2)
            ]
            v_in = [
                dram.tile([1024, A], BF, name=f"v_in{c}", tag=f"vi{c}")
                for c in range(2)
            ]
            v_out = [
                dram.tile([2, 1024, A], BF, name=f"v_out{c}", tag=f"vo{c}")
                for c in range(2)
            ]

            cc_warm_in = dram.tile([1, 128], BF, name="cc_warm_in")
            cc_warm_out = dram.tile([2, 1, 128], BF, name="cc_warm_out")

            v_sb = singles.tile([128, NK, A], BF)        # V resident, 8.4 MB
            qt_all = singles.tile([128, NC, SQ], BF)     # Q^T resident, 4.2 MB
            wo_sb = singles.tile([128, NC, DIM], BF)     # WoT, 2.1 MB
            bqc_sb = singles.tile([128, NA], F32)
            bkc_sb = singles.tile([128, NA], F32)
            bvb_sb = singles.tile([128, A], F32)
            bob_sb = singles.tile([128, DIM], F32)
            ones_k = singles.tile([128, 1], BF)          # sums matmul lhsT
            ones_1 = singles.tile([1, 1], F32)           # row->partition matmul rhs

            # ---------------- Phase 1: projections + K/V exchange ----------
            with (
                tc.tile_pool(name="p1w", bufs=1) as p1w,
                tc.tile_pool(name="p1x", bufs=1) as p1x,
                tc.tile_pool(name="p1o", bufs=4) as p1o,
                tc.tile_pool(name="p1pk", bufs=2, space="PSUM") as p1pk,
                tc.tile_pool(name="p1pv", bufs=2, space="PSUM") as p1pv,
            ):
                wk = p1w.tile([128, NC, A], BF, tag="wkq")
                wv = p1w.tile([128, NC, A], BF, tag="wv")
                # all of x^T stays resident through phase 1 so no PE input
                # depends on DMA while the collectives are saturating HBM
                xs_all = p1x.tile([128, NC, SQ], BF)

                # minimal DMA before the first matmul: wk + first x block,
                # spread across queues so dc=0 matmuls start early
                nc.sync.dma_start(
                    out=wk[:, 0:4, :],
                    in_=WkT[:, :].rearrange("(c p) a -> p c a", p=128)[:, 0:4, :],
                )
                nc.scalar.dma_start(
                    out=wk[:, 4:8, :],
                    in_=WkT[:, :].rearrange("(c p) a -> p c a", p=128)[:, 4:8, :],
                )
                nc.scalar.dma_start(out=bkc_sb[:], in_=bkc[:])
                for sb in range(4):
                    (nc.gpsimd if sb == 0 else nc.sync).dma_start(
                        out=xs_all[:, :, sb * 512 : (sb + 1) * 512],
                        in_=xTq[:, sb * 512 : (sb + 1) * 512].rearrange(
                            "(c p) s -> p c s", p=128
                        ),
                    )

                def kt_chunk(c):
                    for sbl in range(2):
                        sb = c * 2 + sbl
                        for am in range(NA):
                            pk = p1pk.tile([128, 512], F32)
                            for dc in range(NC):
                                nc.tensor.matmul(
                                    pk[:],
                                    lhsT=wk[:, dc, am * 128 : (am + 1) * 128],
                                    rhs=xs_all[:, dc, sb * 512 : (sb + 1) * 512],
                                    start=(dc == 0),
                                    stop=(dc == NC - 1),
                                )
                            ko = p1o.tile([128, 512], BF)
                            nc.scalar.activation(
                                ko[:], pk[:], AF.Identity, bias=bkc_sb[:, am : am + 1]
                            )
                            nc.sync.dma_start(
                                out=kt_in[c][
                                    am * 128 : (am + 1) * 128,
                                    sbl * 512 : (sbl + 1) * 512,
                                ],
                                in_=ko[:],
                            )
                    nc.gpsimd.collective_compute(
                        "AllGather",
                        mybir.AluOpType.bypass,
                        replica_groups=PAIRS,
                        ins=[kt_in[c][:].opt()],
                        outs=[kt_out[c][:].opt()],
                    )

                def v_chunk(c):
                    for sbl in range(2):
                        sb = c * 2 + sbl
                        for st in range(4):
                            pv = p1pv.tile([128, 1024], F32)
                            for half in range(2):
                                for dc in range(NC):
                                    nc.tensor.matmul(
                                        pv[:, half * 512 : (half + 1) * 512],
                                        lhsT=xs_all[:, dc, sb * 512 + st * 128 : sb * 512 + (st + 1) * 128],
                                        rhs=wv[:, dc, half * 512 : (half + 1) * 512],
                                        start=(dc == 0),
                                        stop=(dc == NC - 1),
                                    )
                            vo = p1o.tile([128, 1024], BF, tag="vo")
                            nc.vector.tensor_add(vo[:], pv[:], bvb_sb[:])
                            nc.scalar.dma_start(
                                out=v_in[c][
                                    (sbl * 4 + st) * 128 : (sbl * 4 + st + 1) * 128, :
                                ],
                                in_=vo[:],
                            )
                    nc.gpsimd.collective_compute(
                        "AllGather",
                        mybir.AluOpType.bypass,
                        replica_groups=PAIRS,
                        ins=[v_in[c][:].opt()],
                        outs=[v_out[c][:].opt()],
                    )

                # K^T chunks first so the exchanges start as early as possible
                kt_chunk(0)
                nc.sync.dma_start(out=wv[:], in_=WvT.rearrange("(c p) a -> p c a", p=128))
                nc.scalar.dma_start(out=bvb_sb[:], in_=bvb[:])
                kt_chunk(1)
                wq = p1w.tile([128, NC, A], BF, tag="wkq")
                nc.sync.dma_start(out=wq[:], in_=WqT.rearrange("(c p) a -> p c a", p=128))
                nc.scalar.dma_start(out=bqc_sb[:], in_=bqc[:])
                v_chunk(0)
                v_chunk(1)

                # gathered V -> resident SBUF, k enumerated in gather order
                for c in range(2):
                    for hh in range(2):
                        nc.gpsimd.dma_start(
                            out=v_sb[:, c * 16 + hh * 8 : c * 16 + hh * 8 + 8, :],
                            in_=v_out[c][hh].rearrange("(j p) a -> p j a", p=128),
                        )

                # --- Q projection (overlaps the V exchanges) ---
                for qb in range(NQB):
                    for am in range(NA):
                        pq = p1pk.tile([128, 512], F32)
                        for dc in range(NC):
                            nc.tensor.matmul(
                                pq[:],
                                lhsT=wq[:, dc, am * 128 : (am + 1) * 128],
                                rhs=xs_all[:, dc, qb * 512 : (qb + 1) * 512],
                                start=(dc == 0),
                                stop=(dc == NC - 1),
                            )
                        nc.scalar.activation(
                            qt_all[:, am, qb * 512 : (qb + 1) * 512],
                            pq[:],
                            AF.Identity,
                            bias=bqc_sb[:, am : am + 1],
                        )

                # singles needed only for phase 2
                nc.sync.dma_start(
                    out=wo_sb[:], in_=WoT.rearrange("(c p) d -> p c d", p=128)
                )
                nc.sync.dma_start(out=bob_sb[:], in_=bob[:])
                nc.vector.memset(ones_k[:], 1.0)
                nc.vector.memset(ones_1[:], 1.0)

            # ---------------- Phase 2: attention ----------------
            with (
                tc.tile_pool(name="p2k", bufs=2) as p2k,
                tc.tile_pool(name="p2e", bufs=1) as p2e,
                tc.tile_pool(name="p2c", bufs=1) as p2c,
                tc.tile_pool(name="p2s", bufs=2) as p2s,
                tc.tile_pool(name="p2r", bufs=2) as p2r,
                tc.tile_pool(name="p2o", bufs=2) as p2o,
                tc.tile_pool(name="pps", bufs=2, space="PSUM") as pps,
                tc.tile_pool(name="ppsum", bufs=1, space="PSUM") as ppsum,
                tc.tile_pool(name="ppt", bufs=1, space="PSUM") as ppt,
                tc.tile_pool(name="ppc", bufs=2, space="PSUM") as ppc,
                tc.tile_pool(name="ppo", bufs=2, space="PSUM") as ppo,
            ):
                for qb in range(NQB):
                    et = p2e.tile([128, NK, QB], BF)
                    # scores^T + exp; k-tile groups of 4 share one KT load
                    for c in range(2):
                        for hh in range(2):
                            for half in range(2):
                                ks = p2k.tile([128, NC, 512], BF)
                                nc.sync.dma_start(
                                    out=ks[:],
                                    in_=kt_out[c][
                                        hh, :, half * 512 : (half + 1) * 512
                                    ].rearrange("(cp p) k -> p cp k", p=128),
                                )
                                ebase = c * 16 + hh * 8 + half * 4
                                for kt4 in range(4):
                                    ps = pps.tile([128, QB], F32)
                                    for ac in range(NC):
                                        nc.tensor.matmul(
                                            ps[:],
                                            lhsT=ks[:, ac, kt4 * 128 : (kt4 + 1) * 128],
                                            rhs=qt_all[:, ac, qb * QB : (qb + 1) * QB],
                                            start=(ac == 0),
                                            stop=(ac == NC - 1),
                                        )
                                    nc.scalar.activation(
                                        et[:, ebase + kt4, :],
                                        ps[:],
                                        AF.Exp,
                                        scale=float(SCALE),
                                    )
                    # softmax denominators: ones-row matmul, then row->partition
                    p_row = ppsum.tile([1, QB], F32)
                    for kt in range(NK):
                        nc.tensor.matmul(
                            p_row[:],
                            lhsT=ones_k[:, 0:1],
                            rhs=et[:, kt, :],
                            start=(kt == 0),
                            stop=(kt == NK - 1),
                        )
                    srow = p2s.tile([1, QB], F32)
                    nc.scalar.copy(srow[:], p_row[:])
                    recips = p2r.tile([128, 4], F32)
                    for qi in range(4):
                        ptt = ppt.tile([128, 1], F32)
                        nc.tensor.matmul(
                            ptt[:],
                            lhsT=srow[0:1, qi * 128 : (qi + 1) * 128],
                            rhs=ones_1[0:1, 0:1],
                            start=True,
                            stop=True,
                        )
                        nc.vector.reciprocal(recips[:, qi : qi + 1], ptt[:])
                    # unnormalized ctx^T accumulated over k
                    ct = p2c.tile([128, NA, QB], BF)
                    for at in range(NA):
                        pc = ppc.tile([128, QB], F32)
                        for kt in range(NK):
                            nc.tensor.matmul(
                                pc[:],
                                lhsT=v_sb[:, kt, at * 128 : (at + 1) * 128],
                                rhs=et[:, kt, :],
                                start=(kt == 0),
                                stop=(kt == NK - 1),
                            )
                        nc.vector.tensor_copy(ct[:, at, :], pc[:])
                    # output projection + deferred softmax normalization + bias
                    for qi in range(4):
                        for half in range(2):
                            po = ppo.tile([128, 512], F32)
                            for ac in range(NC):
                                nc.tensor.matmul(
                                    po[:],
                                    lhsT=ct[:, ac, qi * 128 : (qi + 1) * 128],
                                    rhs=wo_sb[:, ac, half * 512 : (half + 1) * 512],
                                    start=(ac == 0),
                                    stop=(ac == NC - 1),
                                )
                            ob = p2o.tile([128, 512], F32)
                            nc.vector.tensor_scalar(
                                ob[:],
                                po[:],
                                recips[:, qi : qi + 1],
                                None,
                                op0=mybir.AluOpType.mult,
                            )
                            nc.vector.tensor_add(
                                ob[:], ob[:], bob_sb[:, half * 512 : (half + 1) * 512]
                            )
                            nc.sync.dma_start(
                                out=out[
                                    (qb * 4 + qi) * 128 : (qb * 4 + qi + 1) * 128,
                                    half * 512 : (half + 1) * 512,
                                ],
                                in_=ob[:],
                            )

    _split_multiwaits(nc)
    return nc


_NC_CACHE = None


def _get_nc():
    global _NC_CACHE
    if _NC_CACHE is None:
        _NC_CACHE = _build()
    return _NC_CACHE


def kernel(x, Wq, bq, Wk, bk, Wv, bv, Wo, bo):
    global LAST_RESULT
    bf16 = ml_dtypes.bfloat16
    x = np.asarray(x, np.float32)

    WqT = np.ascontiguousarray(np.asarray(Wq, np.float32).T).astype(bf16)
    WkT = np.ascontiguousarray(np.asarray(Wk, np.float32).T).astype(bf16)
    WvT = np.ascontiguousarray(np.asarray(Wv, np.float32).T).astype(bf16)
    WoT = np.ascontiguousarray(np.asarray(Wo, np.float32).T).astype(bf16)
    bqc = np.ascontiguousarray(np.asarray(bq, np.float32).reshape(NA, 128).T)
    bkc = np.ascontiguousarray(np.asarray(bk, np.float32).reshape(NA, 128).T)
    bvb = np.ascontiguousarray(np.broadcast_to(np.asarray(bv, np.float32), (128, A)))
    bob = np.ascontiguousarray(np.broadcast_to(np.asarray(bo, np.float32), (128, DIM)))

    in_maps = []
    for c in range(N_CORES):
        b, h = c // 2, c % 2
        xTq = np.ascontiguousarray(x[b, h * SQ : (h + 1) * SQ, :].T).astype(bf16)
        in_maps.append(
            {
                "xTq": xTq,
                "WqT": WqT,
                "WkT": WkT,
                "WvT": WvT,
                "WoT": WoT,
                "bqc": bqc,
                "bkc": bkc,
                "bvb": bvb,
                "bob": bob,
            }
        )

    nc = _get_nc()
    import os

    res = run_bass_kernel_spmd(
        nc,
        in_maps,
        core_ids=list(range(N_CORES)),
        trace=bool(os.environ.get("BASS_TRACE")),
    )
    LAST_RESULT = res

    out_full = np.empty((B, S, DIM), np.float32)
    for c in range(N_CORES):
        b, h = c // 2, c % 2
        out_full[b, h * SQ : (h + 1) * SQ, :] = res.results[c]["out"]
    return out_full
